# revision 8
# baseline (speedup 1.0000x reference)
"""Trainium2 Bass kernel for Controller.predict_pairwise_prob (cumm='sum').

Math (per batch b, with T=512 timesteps, C=32 channels):
    a   = log(coref + overwrite)                       [T, C]
    bb  = log(coref)                                   [T, C]
    cum = cumsum_t log((1-overwrite)*(1-EPS) + EPS)    [T, C]
    out[t1, t2] = logsumexp_c(a[t1] + bb[t2] + cum[t2] - cum[t1]) * (t2 > t1)

Key identity: with u = a - cum, v = bb + cum, and per-row maxes
m1[t1] = max_c u[t1, :], m2[t2] = max_c v[t2, :]:

    out[t1, t2] = log( sum_c exp(u[t1,c]-m1[t1]) * exp(v[t2,c]-m2[t2]) )
                  + m1[t1] + m2[t2]

i.e. a [T,C] x [C,T] matmul in exp space -> log -> rank-1 corrections.
The max shifts keep every exp argument in [-~60, 0], so no overflow /
underflow despite cum reaching ~-170.

Sharding: data-parallel over batch, one batch element per NeuronCore.
"""

import numpy as np

import concourse.bacc as bacc
import concourse.bass as bass
import concourse.tile as tile
from concourse import mybir
from concourse.bass_utils import run_bass_kernel_spmd

EPS = 1e-8
P = 128          # partitions / t-block size
T = 512          # timesteps
C = 32           # channels
NB = T // P      # 4 t-blocks
FP = mybir.dt.float32
FR = mybir.dt.float32r
ALU = mybir.AluOpType
AF = mybir.ActivationFunctionType

_CACHE = {}


def _build():
    nc = bacc.Bacc(
        "TRN2",
        target_bir_lowering=False,
        debug=False,
        enable_asserts=True,
        num_devices=8,
    )

    coref = nc.dram_tensor("coref", [T, C], FP, kind="ExternalInput").ap()
    ow = nc.dram_tensor("ow", [T, C], FP, kind="ExternalInput").ap()
    ident = nc.dram_tensor("ident", [P, P], FP, kind="ExternalInput").ap()
    maskt = nc.dram_tensor("maskt", [P, P], FP, kind="ExternalInput").ap()
    oness = nc.dram_tensor("oness", [C, T], FP, kind="ExternalInput").ap()
    zeros = nc.dram_tensor("zeros", [P, T - P], FP, kind="ExternalInput").ap()
    m2s = nc.dram_tensor("m2s", [P, NB], FP, kind="Internal").ap()
    out = nc.dram_tensor("out", [T, T], FP, kind="ExternalOutput").ap()

    with tile.TileContext(nc) as tc:
        _body(tc, out, coref, ow, ident, maskt, oness, zeros, m2s)

    nc.compile()
    return nc


def _body(tc, out, coref, ow, ident, maskt, oness, zeros, m2s):
    nc = tc.nc
    with (
        tc.tile_pool(name="main", bufs=1) as pool,
        tc.tile_pool(name="pp", bufs=2) as pp,
        tc.tile_pool(name="ps", bufs=1, space="PSUM") as psum,
        tc.tile_pool(name="ps_s", bufs=2, space="PSUM") as psum_s,
    ):
        # ---- constants ----
        ident_t = pool.tile([P, P], FP, tag="ident")
        nc.sync.dma_start(ident_t[:], ident)
        mask_t = pool.tile([P, P], FP, tag="mask")
        nc.sync.dma_start(mask_t[:], maskt)
        ones_t = pool.tile([C, T], FP, tag="oness")
        nc.sync.dma_start(ones_t[:], oness)
        zero_t = pool.tile([P, T - P], FP, tag="zeros")
        nc.sync.dma_start(zero_t[:], zeros)

        # ---- load inputs as [t-in-block(128), block(4) x chan(32)] ----
        cor_t = pool.tile([P, P], FP, tag="cor")
        nc.sync.dma_start(
            cor_t[:].rearrange("p (n c) -> p n c", c=C),
            coref.rearrange("(n p) c -> p n c", p=P),
        )
        ow_t = pool.tile([P, P], FP, tag="ow")
        nc.sync.dma_start(
            ow_t[:].rearrange("p (n c) -> p n c", c=C),
            ow.rearrange("(n p) c -> p n c", p=P),
        )

        # ---- w = log(1 - (1-EPS)*ow), a = log(cor+ow), b = log(cor) ----
        w_t = pool.tile([P, P], FP, tag="w")
        nc.scalar.activation(w_t[:], ow_t[:], AF.Ln, bias=1.0, scale=-(1.0 - EPS))
        ab_t = pool.tile([P, P], FP, tag="ab")
        nc.vector.tensor_add(ab_t[:], cor_t[:], ow_t[:])
        a_t = pool.tile([P, P], FP, tag="a")
        nc.scalar.activation(a_t[:], ab_t[:], AF.Ln)
        b_t = pool.tile([P, P], FP, tag="b")
        nc.scalar.activation(b_t[:], cor_t[:], AF.Ln)

        # ---- cum = cumsum_t(w): transpose to [c, t], one scan, transpose back ----
        wT_ps = psum.tile([C, T], FP, tag="tp")
        for n in range(NB):
            nc.tensor.transpose(
                wT_ps[:, P * n : P * (n + 1)],
                w_t[:, C * n : C * (n + 1)],
                ident_t[:],
            )
        wT_t = pool.tile([C, T], FP, tag="wT")
        nc.vector.tensor_copy(wT_t[:], wT_ps[:])

        cum_ct = pool.tile([C, T], FP, tag="cumct")
        nc.vector.tensor_tensor_scan(
            out=cum_ct[:],
            data0=ones_t[:],
            data1=wT_t[:],
            initial=0.0,
            op0=ALU.mult,
            op1=ALU.add,
        )

        cumT_ps = psum.tile([P, P], FP, tag="tp2")
        for n in range(NB):
            nc.tensor.transpose(
                cumT_ps[:, C * n : C * (n + 1)],
                cum_ct[:, P * n : P * (n + 1)],
                ident_t[:C, :C],
            )
        cum_t = pool.tile([P, P], FP, tag="cum")
        nc.vector.tensor_copy(cum_t[:], cumT_ps[:])

        # ---- u = a - cum, v = b + cum; m1n = -max_c u, m2n = -max_c v ----
        u_t = pool.tile([P, P], FP, tag="u")
        nc.vector.tensor_sub(u_t[:], a_t[:], cum_t[:])
        v_t = pool.tile([P, P], FP, tag="v")
        nc.vector.tensor_add(v_t[:], b_t[:], cum_t[:])

        m1n = pool.tile([P, NB], FP, tag="m1n")
        nc.vector.tensor_reduce(
            m1n[:],
            u_t[:].rearrange("p (n c) -> p n c", c=C),
            axis=mybir.AxisListType.X,
            op=ALU.max,
            negate=True,
        )
        m2n = pool.tile([P, NB], FP, tag="m2n")
        nc.vector.tensor_reduce(
            m2n[:],
            v_t[:].rearrange("p (n c) -> p n c", c=C),
            axis=mybir.AxisListType.X,
            op=ALU.max,
            negate=True,
        )

        # ---- uh = exp(u - m1), vh = exp(v - m2) ----
        u2_t = pool.tile([P, P], FP, tag="u2")
        nc.vector.tensor_add(
            u2_t[:].rearrange("p (n c) -> p n c", c=C),
            u_t[:].rearrange("p (n c) -> p n c", c=C),
            m1n[:, :, None].broadcast_to([P, NB, C]),
        )
        v2_t = pool.tile([P, P], FP, tag="v2")
        nc.vector.tensor_add(
            v2_t[:].rearrange("p (n c) -> p n c", c=C),
            v_t[:].rearrange("p (n c) -> p n c", c=C),
            m2n[:, :, None].broadcast_to([P, NB, C]),
        )
        uh_t = pool.tile([P, P], FP, tag="uh")
        nc.scalar.activation(uh_t[:], u2_t[:], AF.Exp)
        vh_t = pool.tile([P, P], FP, tag="vh")
        nc.scalar.activation(vh_t[:], v2_t[:], AF.Exp)

        # ---- transpose uh, vh -> [c, t] with c on partitions 0..31 ----
        uT_ps = psum.tile([C, T], FP, tag="tp3")
        for n in range(NB):
            nc.tensor.transpose(
                uT_ps[:, P * n : P * (n + 1)],
                uh_t[:, C * n : C * (n + 1)],
                ident_t[:],
            )
        uT_t = pool.tile([C, T], FR, tag="uT")
        nc.vector.tensor_copy(uT_t[:], uT_ps[:])

        vT_ps = psum.tile([C, T], FP, tag="tp4")
        for n in range(NB):
            nc.tensor.transpose(
                vT_ps[:, P * n : P * (n + 1)],
                vh_t[:, C * n : C * (n + 1)],
                ident_t[:],
            )
        vT_t = pool.tile([C, T], FR, tag="vT")
        nc.vector.tensor_copy(vT_t[:], vT_ps[:])

        # ---- m2 row broadcast via DRAM roundtrip ----
        # m2s[p, n] = m2n[p, n]; read back transposed+broadcast as [128, (n p)]
        nc.sync.dma_start(m2s, m2n[:])
        m2bc_t = pool.tile([P, T], FP, tag="m2bc")
        for n in range(NB):
            nc.sync.dma_start(
                m2bc_t[:, P * n : P * (n + 1)].rearrange("q (o p) -> q o p", o=1),
                m2s[:, n : n + 1].transpose([1, 0]).partition_broadcast(P),
            )

        # ---- per t1-block: S = uh_i @ vh^T ; out = (ln S - m1n) - m2n_bc ----
        for i in range(NB):
            lo = P * i
            s_ps = psum_s.tile([P, T], FP, tag="s")
            nc.tensor.matmul(
                s_ps[:, lo:],
                uT_t[:, lo : lo + P],
                vT_t[:, lo:],
                start=True,
                stop=True,
            )
            lns_t = pp.tile([P, T], FP, tag="lns")
            nc.scalar.activation(lns_t[:, lo:], s_ps[:, lo:], AF.Ln)
            o_t = pp.tile([P, T], FP, tag="o")
            nc.vector.scalar_tensor_tensor(
                out=o_t[:, lo:],
                in0=lns_t[:, lo:],
                scalar=m1n[:, i : i + 1],
                in1=m2bc_t[:, lo:],
                op0=ALU.subtract,
                op1=ALU.subtract,
            )
            nc.vector.tensor_mul(o_t[:, lo : lo + P], o_t[:, lo : lo + P], mask_t[:])
            nc.sync.dma_start(out[lo : lo + P, lo:], o_t[:, lo:])
            if i > 0:
                nc.sync.dma_start(out[lo : lo + P, :lo], zero_t[:, :lo])


def _consts():
    ident = np.eye(P, dtype=np.float32)
    # mask[p, q] = 1 where q > p (strict upper triangle of the diagonal block)
    maskt = np.triu(np.ones((P, P), dtype=np.float32), k=1)
    oness = np.ones((C, T), dtype=np.float32)
    zeros = np.zeros((P, T - P), dtype=np.float32)
    return {"ident": ident, "maskt": maskt, "oness": oness, "zeros": zeros}


def kernel(coref: np.ndarray, overwrite: np.ndarray) -> np.ndarray:
    B = coref.shape[0]
    assert coref.shape == (B, T, C) and overwrite.shape == (B, T, C)
    if "nc" not in _CACHE:
        _CACHE["nc"] = _build()
    nc = _CACHE["nc"]
    consts = _consts()
    in_maps = [
        {
            "coref": np.ascontiguousarray(coref[b], dtype=np.float32),
            "ow": np.ascontiguousarray(overwrite[b], dtype=np.float32),
            **consts,
        }
        for b in range(B)
    ]
    res = run_bass_kernel_spmd(nc, in_maps, core_ids=list(range(B)))
    return np.stack([r["out"] for r in res.results], axis=0)


# revision 12
# speedup vs baseline: 3.3411x; 3.3411x over previous
"""Trainium2 Bass kernel for Controller.predict_pairwise_prob (cumm='sum').

Math (per batch b, with T=512 timesteps, C=32 channels):
    a   = log(coref + overwrite)                       [T, C]
    bb  = log(coref)                                   [T, C]
    cum = cumsum_t log((1-overwrite)*(1-EPS) + EPS)    [T, C]
    out[t1, t2] = logsumexp_c(a[t1] + bb[t2] + cum[t2] - cum[t1]) * (t2 > t1)

Key identity: with u = a - cum, v = bb + cum, and per-row maxes
m1[t1] = max_c u[t1, :], m2[t2] = max_c v[t2, :]:

    out[t1, t2] = log( sum_c exp(u[t1,c]-m1[t1]) * exp(v[t2,c]-m2[t2]) )
                  + m1[t1] + m2[t2]

i.e. a [T,C] x [C,T] matmul in exp space -> log -> rank-1 corrections.
The max shifts keep every exp argument in [-~60, 0], so no overflow /
underflow despite cum reaching ~-170.

Sharding: data-parallel over batch, one batch element per NeuronCore.
"""

import numpy as np

import concourse.bacc as bacc
import concourse.bass as bass
import concourse.tile as tile
from concourse import mybir
from concourse.bass_utils import run_bass_kernel_spmd

EPS = 1e-8
P = 128          # partitions / t-block size
T = 512          # timesteps
C = 32           # channels
NB = T // P      # 4 t-blocks
FP = mybir.dt.float32
FR = mybir.dt.float32r
ALU = mybir.AluOpType
AF = mybir.ActivationFunctionType

_CACHE = {}


def _build():
    import concourse.bacc as _bacc_mod
    import concourse.hw_specs as _hw
    _orig_tables = _hw.get_activation_tables
    _only = "natural_log_exp_and_others"

    def _patched(arch):
        tabs = _orig_tables(arch)
        return {k: (v if k == _only else set()) for k, v in tabs.items()}

    _bacc_mod.get_activation_tables = _patched
    nc = bacc.Bacc(
        "TRN2",
        target_bir_lowering=False,
        debug=False,
        enable_asserts=True,
        num_devices=8,
    )

    coref = nc.dram_tensor("coref", [T, C], FP, kind="ExternalInput").ap()
    ow = nc.dram_tensor("ow", [T, C], FP, kind="ExternalInput").ap()
    ident = nc.dram_tensor("ident", [P, P], FP, kind="ExternalInput").ap()
    maskt = nc.dram_tensor("maskt", [P, P], FP, kind="ExternalInput").ap()
    oness = nc.dram_tensor("oness", [C, T], FP, kind="ExternalInput").ap()
    zeros = nc.dram_tensor("zeros", [P, T - P], FP, kind="ExternalInput").ap()
    sel = nc.dram_tensor("sel", [NB, T], FP, kind="ExternalInput").ap()
    out = nc.dram_tensor("out", [T, T], FP, kind="ExternalOutput").ap()

    with tile.TileContext(nc) as tc:
        _body(tc, out, coref, ow, ident, maskt, oness, zeros, sel)

    nc.compile()
    return nc


def _body(tc, out, coref, ow, ident, maskt, oness, zeros, sel):
    nc = tc.nc
    with (
        tc.tile_pool(name="main", bufs=1) as pool,
        tc.tile_pool(name="pp", bufs=2) as pp,
        tc.tile_pool(name="ps", bufs=1, space="PSUM") as psum,
        tc.tile_pool(name="ps_s", bufs=2, space="PSUM") as psum_s,
    ):
        # ---- constants ----
        ident_t = pool.tile([P, P], FP, tag="ident")
        nc.sync.dma_start(ident_t[:], ident)
        mask_t = pool.tile([P, P], FP, tag="mask")
        nc.sync.dma_start(mask_t[:], maskt)
        ones_t = pool.tile([C, T], FP, tag="oness")
        nc.sync.dma_start(ones_t[:], oness)
        zero_t = pool.tile([P, T - P], FP, tag="zeros")
        nc.sync.dma_start(zero_t[:], zeros)
        sel_t = pool.tile([NB, T], FP, tag="sel")
        nc.sync.dma_start(sel_t[:], sel)

        # ---- load inputs as [t-in-block(128), block(4) x chan(32)] ----
        cor_t = pool.tile([P, P], FP, tag="cor")
        nc.sync.dma_start(
            cor_t[:].rearrange("p (n c) -> p n c", c=C),
            coref.rearrange("(n p) c -> p n c", p=P),
        )
        ow_t = pool.tile([P, P], FP, tag="ow")
        nc.sync.dma_start(
            ow_t[:].rearrange("p (n c) -> p n c", c=C),
            ow.rearrange("(n p) c -> p n c", p=P),
        )

        # ---- w = log(1 - (1-EPS)*ow), a = log(cor+ow), b = log(cor) ----
        w_t = pool.tile([P, P], FP, tag="w")
        nc.scalar.activation(w_t[:], ow_t[:], AF.Ln, bias=1.0, scale=-(1.0 - EPS))
        ab_t = pool.tile([P, P], FP, tag="ab")
        nc.vector.tensor_add(ab_t[:], cor_t[:], ow_t[:])
        a_t = pool.tile([P, P], FP, tag="a")
        nc.scalar.activation(a_t[:], ab_t[:], AF.Ln)
        b_t = pool.tile([P, P], FP, tag="b")
        nc.scalar.activation(b_t[:], cor_t[:], AF.Ln)

        # ---- cum = cumsum_t(w): transpose to [c, t], one scan, transpose back ----
        wT_ps = psum.tile([C, T], FP, tag="tp")
        for n in range(NB):
            nc.tensor.transpose(
                wT_ps[:, P * n : P * (n + 1)],
                w_t[:, C * n : C * (n + 1)],
                ident_t[:],
            )
        wT_t = pool.tile([C, T], FP, tag="wT")
        nc.vector.tensor_copy(wT_t[:], wT_ps[:])

        cum_ct = pool.tile([C, T], FP, tag="cumct")
        nc.vector.tensor_tensor_scan(
            out=cum_ct[:],
            data0=ones_t[:],
            data1=wT_t[:],
            initial=0.0,
            op0=ALU.mult,
            op1=ALU.add,
        )

        cumT_ps = psum.tile([P, P], FP, tag="tp2")
        for n in range(NB):
            nc.tensor.transpose(
                cumT_ps[:, C * n : C * (n + 1)],
                cum_ct[:, P * n : P * (n + 1)],
                ident_t[:C, :C],
            )
        cum_t = pool.tile([P, P], FP, tag="cum")
        nc.vector.tensor_copy(cum_t[:], cumT_ps[:])

        # ---- u = a - cum, v = b + cum; m1n = -max_c u, m2n = -max_c v ----
        u_t = pool.tile([P, P], FP, tag="u")
        nc.vector.tensor_sub(u_t[:], a_t[:], cum_t[:])
        v_t = pool.tile([P, P], FP, tag="v")
        nc.vector.tensor_add(v_t[:], b_t[:], cum_t[:])

        m1n = pool.tile([P, NB], FP, tag="m1n")
        nc.vector.tensor_reduce(
            m1n[:],
            u_t[:].rearrange("p (n c) -> p n c", c=C),
            axis=mybir.AxisListType.X,
            op=ALU.max,
            negate=True,
        )
        m2n = pool.tile([P, NB], FP, tag="m2n")
        nc.vector.tensor_reduce(
            m2n[:],
            v_t[:].rearrange("p (n c) -> p n c", c=C),
            axis=mybir.AxisListType.X,
            op=ALU.max,
            negate=True,
        )

        # ---- uh = exp(u - m1), vh = exp(v - m2) ----
        u2_t = pool.tile([P, P], FP, tag="u2")
        nc.vector.tensor_add(
            u2_t[:].rearrange("p (n c) -> p n c", c=C),
            u_t[:].rearrange("p (n c) -> p n c", c=C),
            m1n[:, :, None].broadcast_to([P, NB, C]),
        )
        v2_t = pool.tile([P, P], FP, tag="v2")
        nc.vector.tensor_add(
            v2_t[:].rearrange("p (n c) -> p n c", c=C),
            v_t[:].rearrange("p (n c) -> p n c", c=C),
            m2n[:, :, None].broadcast_to([P, NB, C]),
        )
        uh_t = pool.tile([P, P], FP, tag="uh")
        nc.scalar.activation(uh_t[:], u2_t[:], AF.Exp)
        vh_t = pool.tile([P, P], FP, tag="vh")
        nc.scalar.activation(vh_t[:], v2_t[:], AF.Exp)

        # ---- transpose uh, vh -> [c, t] with c on partitions 0..31 ----
        uT_ps = psum.tile([C, T], FP, tag="tp3")
        for n in range(NB):
            nc.tensor.transpose(
                uT_ps[:, P * n : P * (n + 1)],
                uh_t[:, C * n : C * (n + 1)],
                ident_t[:],
            )
        uT_t = pool.tile([C, T], FR, tag="uT")
        nc.vector.tensor_copy(uT_t[:], uT_ps[:])

        vT_ps = psum.tile([C, T], FP, tag="tp4")
        for n in range(NB):
            nc.tensor.transpose(
                vT_ps[:, P * n : P * (n + 1)],
                vh_t[:, C * n : C * (n + 1)],
                ident_t[:],
            )
        vT_t = pool.tile([C, T], FR, tag="vT")
        nc.vector.tensor_copy(vT_t[:], vT_ps[:])

        # ---- m2 broadcast: m2bc[q, 128n+p] = m2n[p, n], all on-chip ----
        # transpose m2n -> [NB, P]; then per block n a K=NB matmul with a
        # constant selector column (sel[:, block n] = e_n) broadcasts row n
        # of m2nT across all 128 output partitions.
        m2nT_ps = psum.tile([NB, P], FP, tag="tp5")
        nc.tensor.transpose(m2nT_ps[:], m2n[:], ident_t[:])
        m2nT_t = pool.tile([NB, P], FP, tag="m2nT")
        nc.vector.tensor_copy(m2nT_t[:], m2nT_ps[:])
        m2bc_ps = psum.tile([P, T], FP, tag="m2bc")
        for n in range(NB):
            nc.tensor.matmul(
                m2bc_ps[:, P * n : P * (n + 1)],
                sel_t[:, P * n : P * (n + 1)],
                m2nT_t[:],
                start=True,
                stop=True,
            )

        # ---- per t1-block: S = uh_i @ vh^T ; out = (ln S - m1n) - m2n_bc ----
        for i in range(NB):
            lo = P * i
            s_ps = psum_s.tile([P, T], FP, tag="s")
            nc.tensor.matmul(
                s_ps[:, lo:],
                uT_t[:, lo : lo + P],
                vT_t[:, lo:],
                start=True,
                stop=True,
            )
            lns_t = pp.tile([P, T], FP, tag="lns")
            nc.scalar.activation(lns_t[:, lo:], s_ps[:, lo:], AF.Ln)
            o_t = pp.tile([P, T], FP, tag="o")
            nc.vector.scalar_tensor_tensor(
                out=o_t[:, lo:],
                in0=lns_t[:, lo:],
                scalar=m1n[:, i : i + 1],
                in1=m2bc_ps[:, lo:],
                op0=ALU.subtract,
                op1=ALU.subtract,
            )
            nc.vector.tensor_mul(o_t[:, lo : lo + P], o_t[:, lo : lo + P], mask_t[:])
            nc.sync.dma_start(out[lo : lo + P, lo:], o_t[:, lo:])
            if i > 0:
                nc.sync.dma_start(out[lo : lo + P, :lo], zero_t[:, :lo])


def _consts():
    ident = np.eye(P, dtype=np.float32)
    # mask[p, q] = 1 where q > p (strict upper triangle of the diagonal block)
    maskt = np.triu(np.ones((P, P), dtype=np.float32), k=1)
    oness = np.ones((C, T), dtype=np.float32)
    zeros = np.zeros((P, T - P), dtype=np.float32)
    sel = np.kron(np.eye(NB, dtype=np.float32), np.ones((1, P), dtype=np.float32))
    return {"ident": ident, "maskt": maskt, "oness": oness, "zeros": zeros, "sel": sel}


def kernel(coref: np.ndarray, overwrite: np.ndarray) -> np.ndarray:
    B = coref.shape[0]
    assert coref.shape == (B, T, C) and overwrite.shape == (B, T, C)
    if "nc" not in _CACHE:
        _CACHE["nc"] = _build()
    nc = _CACHE["nc"]
    consts = _consts()
    in_maps = [
        {
            "coref": np.ascontiguousarray(coref[b], dtype=np.float32),
            "ow": np.ascontiguousarray(overwrite[b], dtype=np.float32),
            **consts,
        }
        for b in range(B)
    ]
    res = run_bass_kernel_spmd(nc, in_maps, core_ids=list(range(B)))
    return np.stack([r["out"] for r in res.results], axis=0)
